# revision 34
# baseline (speedup 1.0000x reference)
"""BitConv2dInfer on 8 Trainium2 NeuronCores.

Reference computation (per full input):
    x = clip(x, -1, 1)                       # x [32, 256, 56, 56] f32
    y = conv2d(x, w_q, pad=1)                # w_q [256, 256, 3, 3] ternary
    y = y * s + bias                         # per-out-channel affine
Sharding: data-parallel over batch — each of the 8 cores gets 4 images and
the full (tiny) weights; outputs concatenate over batch with no comms.

Device kernel (per core, per image) — fp8 DoubleRow variant:
  - every image's input is DMA'd as 8-row bands alternating across the
    three DMA-capable queues (gpsimd/scalar/sync) in deadline order: the
    queues share a ~350 GB/s aggregate round-robin, and engine sub-queues
    don't preserve issue order, so big monolithic transfers would starve
    deadline-critical bands
  - V clamp writes clip(x) straight to fp8e4m3 into a zero-bordered
    [128, 2, 58, 58] tile (DVE fp8-out runs at full rate; the e4m3
    quantization of the clamped activations keeps the end-to-end max
    relative error at 1.58e-2, under the 2e-2 budget — measured against
    the reference on all 32 images)
  - conv as 9 accumulated DoubleRow PE matmuls per (cout_tile, 8-row group):
    per tap t: psum += sum_ci w[t,ci].T @ x_win[ci] in ONE matmul
    (lhsT [128, 2, 128] fp8, rhs [128, 2, 8, 56] fp8): DoubleRow contracts
    both cin tiles at once, 2x the bf16 MAC rate (157 TF/s)
  - the two cout tiles' groups are INTERLEAVED (co0 g0, co1 g0, co0 g1, ...)
    so each arriving input band feeds two matmul groups — doubling the
    slack between a band's bandwidth-limited arrival and its deadline
  - scalar-engine activation evacuates PSUM with per-partition scale+bias
  - DMA f32 results back out (sync queue for co0, gpsimd+sync for co1; the
    last image drains per-group on alternating queues, with the closing
    groups tapered, so the tail isn't one queue's serialized backlog)

The PE clock gate (HAM) starts at 1.2 GHz and only reaches 2.4 GHz after
~3.4us of sustained activity, so the kernel front-runs dummy matmuls on a
zeroed tile while the first input chunks are in flight.

Weights are host-side transposed to lhsT layout [128 cin, co, (tap, ci), cout]
and cast to fp8e4m3 (exact for ternary values).
"""

import sys

sys.path.insert(0, "/opt/trn_rl_repo")

import ml_dtypes
import numpy as np

import concourse.bass as bass  # noqa: F401  (registers engines)
import concourse.mybir as mybir
import concourse.tile as tile
from concourse import bacc
from concourse.bass_utils import run_bass_kernel_spmd

N, CIN, COUT, H, W = 32, 256, 256, 56, 56
NCORES = 8
NB = N // NCORES          # images per core
HP, WP = H + 2, W + 2     # padded spatial
RG = 8                    # output rows per PSUM chunk (8*56=448 <= 512 f32/bank)
NCH = H // RG             # chunks per image
NCI = CIN // 128          # cin tiles
NCO = COUT // 128         # cout tiles
NTAP = 9
# Input row bands. The three DMA-capable queues (gpsimd/scalar/sync) share
# a ~350 GB/s aggregate with round-robin arbitration, so EVERY image's input
# is issued as 8-row band chunks alternating across all three queues in
# deadline order — each queue's FIFO is then globally deadline-sorted and
# a big prefetch can never starve a deadline-critical band. A 9-row first
# band makes matmul group 0 depend on band 0 alone.
BANDS = [(0, 9), (9, 8), (17, 8), (25, 8), (33, 8), (41, 8), (49, 7)]
# (ci0_queue, ci1_queue) per band, cycling the three queues.
BAND_QUEUES = [
    ("g", "s"), ("y", "g"), ("s", "y"), ("g", "s"),
    ("y", "g"), ("s", "y"), ("g", "s"),
]
N_WARM_MM = 16            # dummy matmuls to lift the HAM clock gate

F8 = mybir.dt.float8e4
DR = mybir.MatmulPerfMode.DoubleRow

_compiled = {}


def _build():
    nc = bacc.Bacc("TRN2", target_bir_lowering=False, debug=False)
    f32, bf16 = mybir.dt.float32, mybir.dt.bfloat16
    x_d = nc.dram_tensor("x", [NB, CIN, H, W], f32, kind="ExternalInput").ap()
    w_d = nc.dram_tensor(
        "w", [128, NCO, NTAP * NCI, 128], F8, kind="ExternalInput"
    ).ap()
    sb_d = nc.dram_tensor("sb", [128, 2 * NCO], f32, kind="ExternalInput").ap()
    o_d = nc.dram_tensor("out", [NB, COUT, H, W], f32, kind="ExternalOutput").ap()

    clamp = dict(op0=mybir.AluOpType.max, op1=mybir.AluOpType.min)

    with tile.TileContext(nc) as tc:
        with (
            tc.tile_pool(name="const", bufs=1) as cpool,
            tc.tile_pool(name="xs", bufs=4) as xspool,
            tc.tile_pool(name="xsc", bufs=3) as xscpool,
            tc.tile_pool(name="xpad", bufs=2) as xppool,
            tc.tile_pool(name="osb", bufs=4) as opool,
            tc.tile_pool(name="ps", bufs=7, space="PSUM") as pspool,
            tc.tile_pool(name="warmps", bufs=1, space="PSUM") as wpspool,
        ):
            w_sb = cpool.tile([128, NCO, NTAP * NCI, 128], F8, tag="w")
            sb_sb = cpool.tile([128, 2 * NCO], f32, tag="sb")

            # HAM pre-warm (memset on gpsimd so the vector engine's queue
            # stays clear for the border memsets + clamps that gate the
            # first real matmul group).
            warm = cpool.tile([128, RG * W], bf16, tag="warm")
            nc.gpsimd.memset(warm[:], 0.0)
            warm_ps = wpspool.tile([128, RG * W], f32, tag="warmps")
            for _ in range(N_WARM_MM):
                nc.tensor.matmul(
                    out=warm_ps[:], lhsT=warm[:, 0:128], rhs=warm[:],
                    start=True, stop=True,
                )

            def border_memsets(t):
                for ci in range(NCI):
                    nc.vector.memset(t[:, ci, 0:1, :], 0.0)
                    nc.vector.memset(t[:, ci, HP - 1:HP, :], 0.0)
                    nc.vector.memset(t[:, ci, 1:HP - 1, 0:1], 0.0)
                    nc.vector.memset(t[:, ci, 1:HP - 1, WP - 1:WP], 0.0)

            # First image, row-banded. Critical set for the first matmul
            # group: the co0 weight tile + 9 input rows of both ci.
            n0_xp = xppool.tile([128, NCI, HP, WP], F8, tag="xp")
            border_memsets(n0_xp)
            engines = {"g": nc.gpsimd, "s": nc.scalar, "y": nc.sync}
            n0_stage = []
            for k, (r0, nr) in enumerate(BANDS):
                for ci in range(NCI):
                    eng_key = BAND_QUEUES[k][ci]
                    xs = xscpool.tile([128, 16, W], f32, tag=f"xsc{eng_key}")
                    engines[eng_key].dma_start(
                        out=xs[:, 0:nr],
                        in_=x_d[0, ci * 128:(ci + 1) * 128, r0:r0 + nr],
                    )
                    n0_stage.append((r0, nr, ci, xs))
                if k == 0:
                    # Weights right behind the first band: with co0/co1
                    # groups interleaved, both cout tiles are needed within
                    # the first two matmul groups.
                    nc.sync.dma_start(out=w_sb[:, 0], in_=w_d[:, 0])
                    nc.sync.dma_start(out=w_sb[:, 1], in_=w_d[:, 1])
            nc.gpsimd.dma_start(out=sb_sb[:], in_=sb_d)
            for r0, nr, ci, xs in n0_stage:
                nc.vector.tensor_scalar(
                    n0_xp[:, ci, r0 + 1:r0 + nr + 1, 1:W + 1],
                    xs[:, 0:nr], -1.0, 1.0, **clamp,
                )

            # Input for image n, issued one image ahead of its use as the
            # same banded chunks (behind image n-1's bands in every queue's
            # FIFO) so the transfer runs behind the previous image's PE work
            # and never outruns a deadline-critical band.
            def issue_input(n):
                tiles = []
                for ci in range(NCI):
                    xs = xspool.tile([128, H, W], f32, tag="xs")
                    tiles.append(xs)
                for k, (r0, nr) in enumerate(BANDS):
                    for ci in range(NCI):
                        eng_key = BAND_QUEUES[k][ci]
                        engines[eng_key].dma_start(
                            out=tiles[ci][:, r0:r0 + nr],
                            in_=x_d[n, ci * 128:(ci + 1) * 128, r0:r0 + nr],
                        )
                return tiles

            pending = None
            for n in range(NB):
                if n == 0:
                    xp = n0_xp
                else:
                    xp = xppool.tile([128, NCI, HP, WP], F8, tag="xp")
                    border_memsets(xp)
                    for ci in range(NCI):
                        nc.vector.tensor_scalar(
                            xp[:, ci, 1:H + 1, 1:W + 1],
                            pending[ci][:], -1.0, 1.0, **clamp,
                        )
                last_img = n == NB - 1
                # co0/co1 groups interleaved: each input band feeds two
                # matmul groups back to back, doubling the slack between an
                # input band's arrival and its group deadline (image 0's
                # band 6 deadline moves from ~23us to ~33us, past the
                # ~350 GB/s aggregate-bandwidth-limited arrival).
                seq = []
                for c in range(NCH - 1 if last_img else NCH):
                    for co in range(NCO):
                        seq.append((co, c * RG, RG))
                if last_img:
                    # Taper the closing groups so the final ACT + DMA are
                    # small and the tail drains fast.
                    seq.append((0, H - 8, 8))
                    seq += [(1, H - 8, 4), (1, H - 4, 2), (1, H - 2, 2)]
                osbs = [
                    opool.tile([128, H, W], f32, tag="osb", name=f"osb{n}_{co}")
                    for co in range(NCO)
                ]
                for gi, (co, g0, gn) in enumerate(seq):
                    # Prefetch the next image from mid-image: late enough
                    # that (with unordered engine sub-queues) it cannot race
                    # image n's own bands, early enough to land well before
                    # image n+1's first group.
                    if gi == 7 and n + 1 < NB:
                        pending = issue_input(n + 1)
                    osb = osbs[co]
                    ps = pspool.tile([128, RG, W], f32, tag="ps")
                    for t in range(NTAP):
                        kh, kw = divmod(t, 3)
                        nc.tensor.matmul(
                            out=ps[:, 0:gn],
                            lhsT=w_sb[:, co, t * NCI:(t + 1) * NCI],
                            rhs=xp[:, :, g0 + kh:g0 + kh + gn, kw:kw + W],
                            start=(t == 0), stop=(t == NTAP - 1),
                            perf_mode=DR,
                        )
                    nc.scalar.activation(
                        out=osb[:, g0:g0 + gn, :], in_=ps[:, 0:gn],
                        func=mybir.ActivationFunctionType.Identity,
                        bias=sb_sb[:, NCO + co:NCO + co + 1],
                        scale=sb_sb[:, co:co + 1],
                    )
                    dst = o_d[n, co * 128:(co + 1) * 128]
                    if last_img:
                        # The whole last image drains per-group on
                        # alternating queues so the tail isn't one queue's
                        # serialized backlog.
                        eng = nc.gpsimd if (g0 // RG) % 2 == co else nc.sync
                        eng.dma_start(
                            out=dst[:, g0:g0 + gn], in_=osb[:, g0:g0 + gn]
                        )
                    elif g0 + gn == H:
                        # This cout tile just finished — drain it. Queue
                        # budget per image (~23.9us of PE): gpsimd carries
                        # ci0-in + most of co1-out, scalar carries ci1-in,
                        # sync carries w + co0-out + the rest.
                        if co == 0:
                            nc.sync.dma_start(out=dst[:, 0:32], in_=osb[:, 0:32])
                            nc.sync.dma_start(out=dst[:, 32:H], in_=osb[:, 32:H])
                        else:
                            nc.gpsimd.dma_start(out=dst[:, 0:36], in_=osb[:, 0:36])
                            nc.sync.dma_start(out=dst[:, 36:H], in_=osb[:, 36:H])

    nc.compile()
    return nc


def _prep_weights(w_q, s, bias):
    # lhsT layout: [cin_k (128 partitions), co, (tap, ci), cout_j] so that
    # w_t[k, co, t*2+ci, j] = w_q[co*128 + j, ci*128 + k, kh, kw]
    w_t = (
        w_q.astype(np.float32)
        .transpose(2, 3, 1, 0)                 # [kh, kw, CIN, COUT]
        .reshape(NTAP, NCI, 128, NCO, 128)     # [tap, ci, k, co, j]
        .transpose(2, 3, 0, 1, 4)              # [k, co, tap, ci, j]
        .reshape(128, NCO, NTAP * NCI, 128)
        .astype(ml_dtypes.float8_e4m3)
    )
    sb_t = np.concatenate(
        [
            np.ascontiguousarray(s.reshape(NCO, 128).T.astype(np.float32)),
            np.ascontiguousarray(bias.reshape(NCO, 128).T.astype(np.float32)),
        ],
        axis=1,
    )
    return w_t, np.ascontiguousarray(sb_t)


def kernel(x, w_q, s, bias):
    if "nc" not in _compiled:
        _compiled["nc"] = _build()
    nc = _compiled["nc"]

    w_t, sb_t = _prep_weights(w_q, s, bias)
    x = np.ascontiguousarray(x, dtype=np.float32)
    core_ids = list(range(NCORES))
    in_maps = [
        {"x": x[i * NB:(i + 1) * NB], "w": w_t, "sb": sb_t}
        for i in core_ids
    ]
    res = run_bass_kernel_spmd(nc, in_maps, core_ids)
    return np.concatenate([res.results[i]["out"] for i in core_ids], axis=0)
